# revision 8
# baseline (speedup 1.0000x reference)
"""Trainium2 Bass kernel for nn_AttentionBlock (GroupNorm -> MHA -> proj + residual).

Contract: kernel(**inputs) takes the FULL unsharded inputs (as produced by
setup_inputs) and returns the FULL output [8, 512, 32, 32] float32.

Sharding: pure data-parallel over batch B=8 across the 8 NeuronCores; each core
processes one batch element end-to-end (no collectives needed).

Per-core design (B=1, C=512, N=H*W=1024, heads=8, head_dim=64), fp8-first:

  All four matmul groups run as float8e4 (E4M3) DoubleRow matmuls at 0.5
  cycles/row (2 k-tiles contracted per instruction):
   - qkv:  h stored fp8 in [128, kpair(2), kslot(2), 1024]; weights host-
     rearranged so each matmul contracts 256 channels.  q/k output channels
     are reordered on host so head h occupies partitions 32*(h%4)..+32 with
     head-dim split across two free-dim slots -> S matmuls can use DoubleRow
     with K=32 x 2 slots.
   - S^T:  per (head, query-half, key-tile): lhsT=k [32,2,128], rhs=q
     [32,2,512] -> S^T [128 keys, 512 queries] fp32 PSUM.
   - softmax: exp(S-3) on ScalarE straight to fp8 E tiles (shift keeps
     max E ~ e^4.2 well below the 240 fp8 max; shift cancels in the
     normalization).  Optionally the first N_SCH heads compute exp on
     VectorE instead via a Schraudolph int16 trick (bf16 bits = round(
     S*128/ln2 + 16250.5)) to offload the ScalarE bottleneck; those heads
     run their AV in bf16 (non-DoubleRow).
   - AV: lhsT = vT blocks [ones(64) | v(64)] per head so PSUM rows 0:64
     hold the softmax denominator (broadcast across partitions) and rows
     64:128 hold A@V; DoubleRow over key-tile pairs.  Epilogue: custom-DVE
     fast reciprocal reads the denominator straight from PSUM (base
     partition 0), one tensor_tensor multiplies+casts O to fp8.
   - proj: DoubleRow over O channel pairs, + (x + pb) residual, DMA out.

  GroupNorm is pipelined per 128-channel tile against the x DMA:
  bn_stats/bn_aggr (DVE), tiny PE matmuls for the group combine/broadcast,
  rsqrt via fast-reciprocal + Newton (DVE), and the normalize runs on
  ScalarE (Identity activation with per-partition scale/bias) writing h
  as fp8 directly.

  v-bias and proj-bias folded on host: pb_eff = proj_b + proj_w @ b_v;
  q scale (1/8) folded into wq/bq on host.
"""

import numpy as np
import ml_dtypes

import concourse.bass as bass
import concourse.tile as tile
from concourse import bacc, mybir
from concourse.bass_utils import run_bass_kernel_spmd

FP32 = mybir.dt.float32
BF16 = mybir.dt.bfloat16
FP8 = mybir.dt.float8e4
I16 = mybir.dt.int16
AF = mybir.ActivationFunctionType
OP = mybir.AluOpType
DR = mybir.MatmulPerfMode.DoubleRow

P = 128      # SBUF partitions
C = 512      # channels
NT = 1024    # spatial tokens (32*32)
NH = 8       # heads
HD = 64      # head dim
NCORES = 8
GSZ = 16     # channels per group (512/32)

# number of leading heads whose exp runs on VectorE (Schraudolph int16)
N_SCH = 0
C_SHIFT = 3.0                     # exp(S - C_SHIFT) on the ScalarE path
A_SCH = 128.0 / float(np.log(2.0))
B_SCH = 127.0 * 128.0 - 5.5       # centered Schraudolph constant (round mode)


def _emit(tc: "tile.TileContext", io: dict):
    nc = tc.nc
    import contextlib
    from collections import deque
    ctx = contextlib.ExitStack()
    with ctx:
        pers = ctx.enter_context(tc.tile_pool(name="pers", bufs=1))
        sm = ctx.enter_context(tc.tile_pool(name="small", bufs=1))

        # ---------------- input DMAs ----------------
        x_sb = pers.tile([P, 4, NT], FP32, tag="x")
        dmae = [nc.sync, nc.gpsimd, nc.scalar, nc.sync]
        for r in range(4):
            dmae[r].dma_start(x_sb[:, r, :], io["x"][:, r, :])
        amat_sb = pers.tile([P, NH], FP32, tag="amat")
        nc.scalar.dma_start(amat_sb, io["amat"])
        imat_sb = pers.tile([NH, P], FP32, tag="imat")
        nc.scalar.dma_start(imat_sb, io["imat"])
        ggc_sb = pers.tile([P, 4], FP32, tag="ggc")
        nc.scalar.dma_start(ggc_sb, io["ggc"])
        gbc_sb = pers.tile([P, 4], FP32, tag="gbc")
        nc.scalar.dma_start(gbc_sb, io["gbc"])
        bqc_sb = pers.tile([P, 4], FP32, tag="bqc")
        nc.scalar.dma_start(bqc_sb, io["bqc"])
        bkc_sb = pers.tile([P, 4], FP32, tag="bkc")
        nc.scalar.dma_start(bkc_sb, io["bkc"])
        pbc_sb = pers.tile([P, 4], FP32, tag="pbc")
        nc.scalar.dma_start(pbc_sb, io["pbc"])
        # weights: k first (first consumer), then q, v, proj
        wk8_sb = pers.tile([P, 2, 2, 4, P], FP8, tag="wk8")
        nc.sync.dma_start(wk8_sb, io["wk8"])
        wq8_sb = pers.tile([P, 2, 2, 4, P], FP8, tag="wq8")
        nc.gpsimd.dma_start(wq8_sb, io["wq8"])
        wv8_sb = pers.tile([P, 2, 2, C], FP8, tag="wv8")
        nc.sync.dma_start(wv8_sb, io["wv8"])
        pw8_sb = pers.tile([P, 2, 2, 4, P], FP8, tag="pw8")
        nc.gpsimd.dma_start(pw8_sb, io["pw8"])

        # preload the exp activation table while DMAs are in flight
        warm_sb = pers.tile([1, 1], FP32, tag="actwarm")
        nc.vector.memset(warm_sb, 0.0)
        nc.scalar.activation(warm_sb, warm_sb, AF.Exp)
        nbias = pers.tile([P, 1], FP32, tag="nbias")
        nc.vector.memset(nbias, -C_SHIFT)

        # persistent activation tensors
        h8_sb = pers.tile([P, 2, 2, NT], FP8, tag="h8")
        q8_sb = pers.tile([P, 4, NT], FP8, tag="q8")
        k8_sb = pers.tile([P, 4, NT], FP8, tag="k8")
        O8_sb = pers.tile([P, 2, 2, NT], FP8, tag="O8")
        vT8_sb = pers.tile([P, 8, NH, P], FP8, tag="vT8")
        if N_SCH:
            vT16_sb = pers.tile([P, 8, N_SCH, P], BF16, tag="vT16")
            nc.gpsimd.memset(vT16_sb[:, :, :, 0:HD], 1.0)
        nc.gpsimd.memset(vT8_sb[:, :, N_SCH:NH, 0:HD], 1.0)
        xpb_sb = pers.tile([P, 4, NT], FP32, tag="xpb")

        # ---------------- GroupNorm (per-tile pipeline) ----------------
        # 16-channel groups never cross a 128-channel tile; each tile is
        # normalized as soon as its x DMA lands: bn_stats/aggr on DVE, group
        # combine + broadcast via tiny PE matmuls, rsqrt = fast-recip + 2
        # Newton steps (DVE), normalize on ScalarE (Identity w/ per-partition
        # scale+bias) writing fp8 h directly.
        with nc.named_scope("gn"), \
             tc.tile_pool(name="gnps", bufs=2, space="PSUM") as gnps:
            for r in range(4):
                st = sm.tile([P, 2, 6], FP32, tag=f"bnstats{r}")
                nc.vector.bn_stats(st[:, 0, :], x_sb[:, r, 0:512])
                nc.vector.bn_stats(st[:, 1, :], x_sb[:, r, 512:1024])
                mv = sm.tile([P, 2], FP32, tag=f"mv{r}")
                nc.vector.bn_aggr(mv, st)
                # (mean, E[x^2]) per channel
                st2 = sm.tile([P, 2], FP32, tag=f"st2{r}")
                nc.vector.tensor_copy(st2[:, 0:1], mv[:, 0:1])
                nc.vector.tensor_tensor(st2[:, 1:2], mv[:, 0:1], mv[:, 0:1],
                                        OP.mult)
                nc.vector.tensor_tensor(st2[:, 1:2], st2[:, 1:2], mv[:, 1:2],
                                        OP.add)
                # per-group (mean, E[x^2]) via PE combine
                G_ps = gnps.tile([NH, 2], FP32, tag="gps", name=f"gps{r}")
                nc.tensor.matmul(G_ps, amat_sb, st2, start=True, stop=True)
                stg = sm.tile([NH, 2], FP32, tag=f"stg{r}")
                nc.vector.tensor_copy(stg, G_ps)
                var = sm.tile([NH, 1], FP32, tag=f"var{r}")
                nc.vector.tensor_tensor(var, stg[:, 0:1], stg[:, 0:1], OP.mult)
                nc.vector.tensor_tensor(var, stg[:, 1:2], var, OP.subtract)
                nc.vector.tensor_scalar(var, var, 1e-5, None, OP.add)
                # rstd = rsqrt(var): 1/var seed + 2 Newton steps
                y = sm.tile([NH, 1], FP32, tag=f"rsy{r}")
                nc.vector.reciprocal_approx_fast(y, var)
                t_ = sm.tile([NH, 1], FP32, tag=f"rst{r}")
                for it in range(2):
                    nc.vector.tensor_tensor(t_, y, y, OP.mult)
                    nc.vector.tensor_tensor(t_, t_, var, OP.mult)
                    nc.vector.tensor_scalar(t_, t_, -0.5, 1.5, OP.mult, OP.add)
                    if it < 1:
                        nc.vector.tensor_tensor(y, y, t_, OP.mult)
                    else:
                        nc.vector.tensor_tensor(stg[:, 1:2], y, t_, OP.mult)
                # broadcast (mean, rstd) back to channels
                MR_ps = gnps.tile([P, 2], FP32, tag="mrps", name=f"mrps{r}")
                nc.tensor.matmul(MR_ps, imat_sb, stg, start=True, stop=True)
                mr = sm.tile([P, 2], FP32, tag=f"mr{r}")
                nc.vector.tensor_copy(mr, MR_ps)
                a_r = sm.tile([P, 1], FP32, tag=f"gn_a{r}")
                nc.vector.tensor_tensor(a_r, mr[:, 1:2], ggc_sb[:, r:r + 1],
                                        OP.mult)
                b_r = sm.tile([P, 1], FP32, tag=f"gn_b{r}")
                nc.vector.tensor_tensor(b_r, mr[:, 0:1], a_r, OP.mult)
                nc.vector.tensor_tensor(b_r, gbc_sb[:, r:r + 1], b_r,
                                        OP.subtract)
                nc.scalar.activation(h8_sb[:, r // 2, r % 2, :], x_sb[:, r, :],
                                     AF.Identity, bias=b_r, scale=a_r)

        # ------------- qkv + attention + proj (fp8 DoubleRow) -------------
        with nc.named_scope("attn"), \
             tc.tile_pool(name="bgps", bufs=2, space="PSUM") as bgps, \
             tc.tile_pool(name="spool", bufs=2, space="PSUM") as spool, \
             tc.tile_pool(name="opool", bufs=2, space="PSUM") as opool, \
             tc.tile_pool(name="epool", bufs=4) as epool, \
             tc.tile_pool(name="rpool", bufs=2) as rpool, \
             tc.tile_pool(name="outp", bufs=4) as outp:

            def qk_chain(dst8, w_sb, bcol, r, half):
                hs = 512 * half
                ps = bgps.tile([P, 512], FP32, tag="bg",
                               name=f"qk_{w_sb.name}_{r}_{half}")
                for kpr in range(2):
                    nc.tensor.matmul(ps, w_sb[:, kpr, :, r, :],
                                     h8_sb[:, kpr, :, hs:hs + 512],
                                     start=(kpr == 0), stop=(kpr == 1),
                                     perf_mode=DR)
                nc.vector.tensor_scalar(dst8[:, r, hs:hs + 512], ps,
                                        bcol[:, r:r + 1], None, OP.add)

            def vt_chain(t):
                ps = bgps.tile([P, 512], FP32, tag="bg", name=f"vt{t}")
                for kpr in range(2):
                    nc.tensor.matmul(ps, h8_sb[:, kpr, :, P * t:P * t + P],
                                     wv8_sb[:, kpr, :, :],
                                     start=(kpr == 0), stop=(kpr == 1),
                                     perf_mode=DR)
                psv = ps.rearrange("p (h c) -> p h c", c=HD)
                if N_SCH:
                    nc.vector.tensor_copy(vT16_sb[:, t, :, HD:P],
                                          psv[:, 0:N_SCH, :])
                nc.vector.tensor_copy(vT8_sb[:, t, N_SCH:NH, HD:P],
                                      psv[:, N_SCH:NH, :])

            def xpb_task(r):
                nc.gpsimd.tensor_scalar(xpb_sb[:, r, :], x_sb[:, r, :],
                                        pbc_sb[:, r:r + 1], None, OP.add)

            def proj_chain(r, half):
                hs = 512 * half
                ps = bgps.tile([P, 512], FP32, tag="bg", name=f"pj{r}_{half}")
                for opr in range(2):
                    nc.tensor.matmul(ps, pw8_sb[:, opr, :, r, :],
                                     O8_sb[:, opr, :, hs:hs + 512],
                                     start=(opr == 0), stop=(opr == 1),
                                     perf_mode=DR)
                o_sb = outp.tile([P, 512], FP32, tag="osb",
                                 name=f"osb{r}_{half}")
                nc.vector.tensor_tensor(o_sb, ps, xpb_sb[:, r, hs:hs + 512],
                                        OP.add)
                eng = nc.sync if (r + half) % 2 == 0 else nc.gpsimd
                eng.dma_start(io["out"][:, r, hs:hs + 512], o_sb)

            # upfront: what head 0 (half 0) needs
            qk_chain(k8_sb, wk8_sb, bkc_sb, 0, 0)
            qk_chain(k8_sb, wk8_sb, bkc_sb, 0, 1)
            qk_chain(q8_sb, wq8_sb, bqc_sb, 0, 0)

            drip = {
                0: [(vt_chain, (0,)), (vt_chain, (1,))],
                1: [(vt_chain, (2,)), (vt_chain, (3,))],
                2: [(vt_chain, (4,)), (vt_chain, (5,))],
                3: [(vt_chain, (6,)), (vt_chain, (7,))],
                4: [(qk_chain, (k8_sb, wk8_sb, bkc_sb, 1, 0)),
                    (qk_chain, (k8_sb, wk8_sb, bkc_sb, 1, 1))],
                6: [(qk_chain, (q8_sb, wq8_sb, bqc_sb, 1, 0))],
                10: [(qk_chain, (k8_sb, wk8_sb, bkc_sb, 2, 0)),
                     (qk_chain, (k8_sb, wk8_sb, bkc_sb, 2, 1))],
                12: [(qk_chain, (q8_sb, wq8_sb, bqc_sb, 2, 0))],
                14: [(xpb_task, (0,))],
                18: [(qk_chain, (k8_sb, wk8_sb, bkc_sb, 3, 0)),
                     (qk_chain, (k8_sb, wk8_sb, bkc_sb, 3, 1))],
                20: [(qk_chain, (q8_sb, wq8_sb, bqc_sb, 3, 0))],
                22: [(xpb_task, (1,))],
                26: [(qk_chain, (q8_sb, wq8_sb, bqc_sb, 0, 1))],
                28: [(qk_chain, (q8_sb, wq8_sb, bqc_sb, 1, 1)),
                     (xpb_task, (2,))],
                30: [(qk_chain, (q8_sb, wq8_sb, bqc_sb, 2, 1)),
                     (xpb_task, (3,))],
                33: [(qk_chain, (q8_sb, wq8_sb, bqc_sb, 3, 1))],
                36: [(proj_chain, (0, 0))],
                38: [(proj_chain, (1, 0))],
                40: [(proj_chain, (2, 0))],
                42: [(proj_chain, (3, 0))],
            }

            O_ps_map = {}

            # a "group" is one key-tile PAIR of one (head, query-half):
            # 2 S matmuls -> 1 exp instr -> 1 DoubleRow AV matmul.
            # g = half*32 + h*4 + tq, tq in 0..3.
            def s_group(g):
                half, h, tq = g // 32, (g % 32) // 4, g % 4
                r, hi = h // 2, h % 2
                S2 = spool.tile([P, 2, 512], FP32, tag="s2", name=f"s2_{g}")
                for j in range(2):
                    t = 2 * tq + j
                    nc.tensor.matmul(
                        S2[:, j, :],
                        k8_sb[HD * hi:HD * hi + HD, r, P * t:P * t + P],
                        q8_sb[HD * hi:HD * hi + HD, r,
                              512 * half:512 * half + 512],
                        start=True, stop=True)
                if h < N_SCH:
                    E = epool.tile([P, 2, 512], I16, tag="e16", name=f"e_{g}")
                    nc.vector.tensor_scalar(E, S2, A_SCH, B_SCH,
                                            OP.mult, OP.add)
                else:
                    E = epool.tile([P, 2, 512], FP8, tag="e8", name=f"e_{g}")
                    nc.scalar.activation(E, S2, AF.Exp, bias=nbias)
                return E

            def av_group(g, E):
                half, h, tq = g // 32, (g % 32) // 4, g % 4
                if tq == 0:
                    O_ps_map[(h, half)] = opool.tile(
                        [P, 512], FP32, tag="o", name=f"o_{h}_{half}")
                O_ps = O_ps_map[(h, half)]
                if h < N_SCH:
                    Ebf = E.bitcast(BF16)
                    for j in range(2):
                        t = 2 * tq + j
                        nc.tensor.matmul(O_ps, vT16_sb[:, t, h, :],
                                         Ebf[:, j, :],
                                         start=(t == 0), stop=(t == 7))
                else:
                    nc.tensor.matmul(
                        O_ps, vT8_sb[:, 2 * tq:2 * tq + 2, h, :], E,
                        start=(tq == 0), stop=(tq == 3), perf_mode=DR)
                if tq == 3:
                    epilogue(h, half)

            def epilogue(h, half):
                O_ps = O_ps_map.pop((h, half))
                Rh = rpool.tile([HD, 512], FP32, tag="rh",
                                name=f"rh{h}_{half}")
                nc.vector.reciprocal_approx_fast(Rh, O_ps[0:HD, :])
                p0 = HD * (h % 2)
                nc.vector.tensor_tensor(
                    O8_sb[p0:p0 + HD, h // 4, (h % 4) // 2,
                          512 * half:512 * half + 512],
                    O_ps[HD:P, :], Rh, OP.mult)

            pend = deque()
            for g in range(64):
                E = s_group(g)
                pend.append((g, E))
                while len(pend) > 1:
                    av_group(*pend.popleft())
                for fn, args in drip.pop(g, ()):
                    fn(*args)
            while pend:
                av_group(*pend.popleft())
            assert not drip

            with nc.named_scope("proj_tail"):
                for r in range(4):
                    proj_chain(r, 1)


_CACHE: dict = {}


def _build():
    if "nc" in _CACHE:
        return _CACHE["nc"]
    nc = bacc.Bacc("TRN2", target_bir_lowering=False, debug=False,
                   num_devices=NCORES)
    io = {
        "x": nc.dram_tensor("x", [P, 4, NT], FP32, kind="ExternalInput").ap(),
        "wq8": nc.dram_tensor("wq8", [P, 2, 2, 2, 2, P], FP8,
                              kind="ExternalInput").ap(),
        "wk8": nc.dram_tensor("wk8", [P, 2, 2, 2, 2, P], FP8,
                              kind="ExternalInput").ap(),
        "wv8": nc.dram_tensor("wv8", [P, 2, 2, C], FP8,
                              kind="ExternalInput").ap(),
        "pw8": nc.dram_tensor("pw8", [P, 2, 2, 4, P], FP8,
                              kind="ExternalInput").ap(),
        "bqc": nc.dram_tensor("bqc", [P, 4], FP32, kind="ExternalInput").ap(),
        "bkc": nc.dram_tensor("bkc", [P, 4], FP32, kind="ExternalInput").ap(),
        "pbc": nc.dram_tensor("pbc", [P, 4], FP32, kind="ExternalInput").ap(),
        "ggc": nc.dram_tensor("ggc", [P, 4], FP32, kind="ExternalInput").ap(),
        "gbc": nc.dram_tensor("gbc", [P, 4], FP32, kind="ExternalInput").ap(),
        "amat": nc.dram_tensor("amat", [P, NH], FP32,
                               kind="ExternalInput").ap(),
        "imat": nc.dram_tensor("imat", [NH, P], FP32,
                               kind="ExternalInput").ap(),
        "out": nc.dram_tensor("out", [P, 4, NT], FP32,
                              kind="ExternalOutput").ap(),
    }
    with tile.TileContext(nc) as tc:
        _emit(tc, io)
    nc.compile()
    _CACHE["nc"] = nc
    return nc


def _host_prep(inputs):
    x = np.ascontiguousarray(np.asarray(inputs["x"], dtype=np.float32))
    qkv_w = np.asarray(inputs["qkv_w"], dtype=np.float32)
    qkv_b = np.asarray(inputs["qkv_b"], dtype=np.float32)
    proj_w = np.asarray(inputs["proj_w"], dtype=np.float32)
    proj_b = np.asarray(inputs["proj_b"], dtype=np.float32)
    gn_scale = np.asarray(inputs["gn_scale"], dtype=np.float32)
    gn_bias = np.asarray(inputs["gn_bias"], dtype=np.float32)

    s = np.float32(1.0 / np.sqrt(HD))
    f8 = ml_dtypes.float8_e4m3

    def qk_weight(W):
        # -> [kpart(128), kpr(2), ksl(2), r(4), m(128)]; plain row order
        Wt = np.ascontiguousarray(W.T)              # [kc, oc]
        return Wt.reshape(2, 2, P, 4, P).transpose(2, 0, 1, 3, 4)

    def qk_bias(b):
        # -> [p(128), r(4)]
        return np.ascontiguousarray(b.reshape(4, P).T)

    Wq = qkv_w[0:C] * s
    Wk = qkv_w[C:2 * C]
    Wv = qkv_w[2 * C:3 * C]

    # wv: [kpart, kpr, ksl, oc(512)]
    wv8 = np.ascontiguousarray(Wv.T).reshape(2, 2, P, C).transpose(2, 0, 1, 3)

    # pw: O channel oc -> (opart, opr, osl): h = 4*opr + 2*osl + opart//64,
    # c = opart%64; lhsT[k=oc, m=o]: pw8[opart, opr, osl, r, m]
    PwT = np.ascontiguousarray(proj_w.T)            # [oc, o]
    pw8 = PwT.reshape(2, 2, 2, HD, 4, P).transpose(2, 3, 0, 1, 4, 5) \
        .reshape(P, 2, 2, 4, P)

    pb = (proj_b + proj_w @ qkv_b[2 * C:3 * C]).astype(np.float32)

    shared = {
        "wq8": np.ascontiguousarray(qk_weight(Wq)).astype(f8),
        "wk8": np.ascontiguousarray(qk_weight(Wk)).astype(f8),
        "wv8": np.ascontiguousarray(wv8).astype(f8),
        "pw8": np.ascontiguousarray(pw8).astype(f8),
        "bqc": qk_bias((qkv_b[0:C] * s).astype(np.float32)),
        "bkc": qk_bias(qkv_b[C:2 * C].astype(np.float32)),
        "pbc": np.ascontiguousarray(pb.reshape(4, P).T),
        "ggc": np.ascontiguousarray(gn_scale.reshape(4, P).T),
        "gbc": np.ascontiguousarray(gn_bias.reshape(4, P).T),
        # amat: [128, 8], 1/16 where channel p belongs to group j of its tile
        "amat": (np.kron(np.eye(NH, dtype=np.float32),
                         np.ones((GSZ, 1), np.float32)) / GSZ),
        # imat: [8, 128], 1.0 where channel p belongs to group j of its tile
        "imat": np.ascontiguousarray(np.kron(np.eye(NH, dtype=np.float32),
                                             np.ones((1, GSZ), np.float32))),
    }
    B = x.shape[0]
    in_maps = []
    for b in range(B):
        m = dict(shared)
        m["x"] = np.ascontiguousarray(
            x[b].reshape(4, P, NT).transpose(1, 0, 2))
        in_maps.append(m)
    return in_maps


def run(inputs, trace=False):
    nc = _build()
    in_maps = _host_prep(inputs)
    res = run_bass_kernel_spmd(nc, in_maps, list(range(NCORES)), trace=trace)
    out = np.stack([res.results[i]["out"] for i in range(NCORES)], axis=0)
    # [B, 128, 4, 1024] -> [B, 512, 32, 32]
    out = out.transpose(0, 2, 1, 3).reshape(len(in_maps), C, 32, 32)
    return out, res


def kernel(**inputs) -> np.ndarray:
    out, _ = run(inputs, trace=False)
    return out.astype(np.float32)


# revision 11
# speedup vs baseline: 1.1894x; 1.1894x over previous
"""Trainium2 Bass kernel for nn_AttentionBlock (GroupNorm -> MHA -> proj + residual).

Contract: kernel(**inputs) takes the FULL unsharded inputs (as produced by
setup_inputs) and returns the FULL output [8, 512, 32, 32] float32.

Sharding: pure data-parallel over batch B=8 across the 8 NeuronCores; each core
processes one batch element end-to-end (no collectives needed).

Per-core design (B=1, C=512, N=H*W=1024, heads=8, head_dim=64), fp8-first:

  All four matmul groups run as float8e4 (E4M3) DoubleRow matmuls at 0.5
  cycles/row (2 k-tiles contracted per instruction):
   - qkv:  h stored fp8 in [128, kpair(2), kslot(2), 1024]; weights host-
     rearranged so each matmul contracts 256 channels.  q/k output channels
     are reordered on host so head h occupies partitions 32*(h%4)..+32 with
     head-dim split across two free-dim slots -> S matmuls can use DoubleRow
     with K=32 x 2 slots.
   - S^T:  per (head, query-half, key-tile): lhsT=k [32,2,128], rhs=q
     [32,2,512] -> S^T [128 keys, 512 queries] fp32 PSUM.
   - softmax: exp(S-3) on ScalarE straight to fp8 E tiles (shift keeps
     max E ~ e^4.2 well below the 240 fp8 max; shift cancels in the
     normalization).  Optionally the first N_SCH heads compute exp on
     VectorE instead via a Schraudolph int16 trick (bf16 bits = round(
     S*128/ln2 + 16250.5)) to offload the ScalarE bottleneck; those heads
     run their AV in bf16 (non-DoubleRow).
   - AV: lhsT = vT blocks [ones(64) | v(64)] per head so PSUM rows 0:64
     hold the softmax denominator (broadcast across partitions) and rows
     64:128 hold A@V; DoubleRow over key-tile pairs.  Epilogue: custom-DVE
     fast reciprocal reads the denominator straight from PSUM (base
     partition 0), one tensor_tensor multiplies+casts O to fp8.
   - proj: DoubleRow over O channel pairs, + (x + pb) residual, DMA out.

  GroupNorm is pipelined per 128-channel tile against the x DMA:
  bn_stats/bn_aggr (DVE), tiny PE matmuls for the group combine/broadcast,
  rsqrt via fast-reciprocal + Newton (DVE), and the normalize runs on
  ScalarE (Identity activation with per-partition scale/bias) writing h
  as fp8 directly.

  v-bias and proj-bias folded on host: pb_eff = proj_b + proj_w @ b_v;
  q scale (1/8) folded into wq/bq on host.
"""

import numpy as np
import ml_dtypes

import concourse.bass as bass
import concourse.tile as tile
from concourse import bacc, mybir
from concourse.bass_utils import run_bass_kernel_spmd

FP32 = mybir.dt.float32
BF16 = mybir.dt.bfloat16
FP8 = mybir.dt.float8e4
I16 = mybir.dt.int16
AF = mybir.ActivationFunctionType
OP = mybir.AluOpType
DR = mybir.MatmulPerfMode.DoubleRow

P = 128      # SBUF partitions
C = 512      # channels
NT = 1024    # spatial tokens (32*32)
NH = 8       # heads
HD = 64      # head dim
NCORES = 8
GSZ = 16     # channels per group (512/32)

# (head, key-tile-quad) groups whose exp runs on VectorE (Schraudolph int16)
# instead of ScalarE; their AV runs bf16 non-DoubleRow from vT16.
SCH_SET = frozenset([(0, 0), (0, 1), (0, 2), (0, 3), (1, 0)])
SCH_HEADS = tuple(sorted({h for h, _ in SCH_SET}))          # need vT16 blocks
VT8_H0 = 1 if (0, 0) in SCH_SET and all((0, t) in SCH_SET for t in range(4))     else 0                                                   # heads needing vT8
C_SHIFT = 3.0                     # exp(S - C_SHIFT) on the ScalarE path
A_SCH = 128.0 / float(np.log(2.0))
# centered Schraudolph constant (round-to-nearest convert), with the same
# -C_SHIFT folded in as the ScalarE exp path so mixed heads stay consistent
B_SCH = 127.0 * 128.0 - 5.5 - A_SCH * C_SHIFT


def _emit(tc: "tile.TileContext", io: dict):
    nc = tc.nc
    import contextlib
    from collections import deque
    ctx = contextlib.ExitStack()
    with ctx:
        pers = ctx.enter_context(tc.tile_pool(name="pers", bufs=1))
        sm = ctx.enter_context(tc.tile_pool(name="small", bufs=1))

        # ---------------- input DMAs ----------------
        x_sb = pers.tile([P, 4, NT], FP32, tag="x")
        dmae = [nc.sync, nc.gpsimd, nc.scalar, nc.sync]
        for r in range(4):
            dmae[r].dma_start(x_sb[:, r, :], io["x"][:, r, :])
        amat_sb = pers.tile([P, NH], FP32, tag="amat")
        nc.scalar.dma_start(amat_sb, io["amat"])
        imat_sb = pers.tile([NH, P], FP32, tag="imat")
        nc.scalar.dma_start(imat_sb, io["imat"])
        ggc_sb = pers.tile([P, 4], FP32, tag="ggc")
        nc.scalar.dma_start(ggc_sb, io["ggc"])
        gbc_sb = pers.tile([P, 4], FP32, tag="gbc")
        nc.scalar.dma_start(gbc_sb, io["gbc"])
        bqc_sb = pers.tile([P, 4], FP32, tag="bqc")
        nc.scalar.dma_start(bqc_sb, io["bqc"])
        bkc_sb = pers.tile([P, 4], FP32, tag="bkc")
        nc.scalar.dma_start(bkc_sb, io["bkc"])
        pbc_sb = pers.tile([P, 4], FP32, tag="pbc")
        nc.scalar.dma_start(pbc_sb, io["pbc"])
        # weights: k first (first consumer), then q, v, proj
        wk8_sb = pers.tile([P, 2, 2, 2, 2, P], FP8, tag="wk8")
        nc.sync.dma_start(wk8_sb, io["wk8"])
        wq8_sb = pers.tile([P, 2, 2, 2, 2, P], FP8, tag="wq8")
        nc.gpsimd.dma_start(wq8_sb, io["wq8"])
        wv8_sb = pers.tile([P, 2, 2, C], FP8, tag="wv8")
        nc.sync.dma_start(wv8_sb, io["wv8"])
        pw8_sb = pers.tile([P, 2, 2, 4, P], FP8, tag="pw8")
        nc.gpsimd.dma_start(pw8_sb, io["pw8"])

        # preload the exp activation table while DMAs are in flight
        warm_sb = pers.tile([1, 1], FP32, tag="actwarm")
        nc.vector.memset(warm_sb, 0.0)
        nc.scalar.activation(warm_sb, warm_sb, AF.Exp)
        nbias = pers.tile([P, 1], FP32, tag="nbias")
        nc.vector.memset(nbias, -C_SHIFT)

        # persistent activation tensors
        h8_sb = pers.tile([P, 2, 2, NT], FP8, tag="h8")
        q8_sb = pers.tile([P, 2, 2, NT], FP8, tag="q8")
        k8_sb = pers.tile([P, 2, 2, NT], FP8, tag="k8")
        O8_sb = pers.tile([P, 2, 2, NT], FP8, tag="O8")
        vT8_sb = pers.tile([P, 8, NH, P], FP8, tag="vT8")
        if SCH_HEADS:
            vT16_sb = pers.tile([P, 8, len(SCH_HEADS), P], BF16, tag="vT16")
            nc.gpsimd.memset(vT16_sb[:, :, :, 0:HD], 1.0)
        nc.gpsimd.memset(vT8_sb[:, :, VT8_H0:NH, 0:HD], 1.0)
        xpb_sb = pers.tile([P, 4, NT], FP32, tag="xpb")
        P1x_sb = pers.tile([P, 4, NT], FP32, tag="p1x")

        # ---------------- GroupNorm (per-tile pipeline) ----------------
        # 16-channel groups never cross a 128-channel tile; each tile is
        # normalized as soon as its x DMA lands: bn_stats/aggr on DVE, group
        # combine + broadcast via tiny PE matmuls, rsqrt = fast-recip + 2
        # Newton steps (DVE), normalize on ScalarE (Identity w/ per-partition
        # scale+bias) writing fp8 h directly.
        with nc.named_scope("gn"), \
             tc.tile_pool(name="gnps", bufs=2, space="PSUM") as gnps:
            for r in range(4):
                st = sm.tile([P, 2, 6], FP32, tag=f"bnstats{r}")
                nc.vector.bn_stats(st[:, 0, :], x_sb[:, r, 0:512])
                nc.vector.bn_stats(st[:, 1, :], x_sb[:, r, 512:1024])
                mv = sm.tile([P, 2], FP32, tag=f"mv{r}")
                nc.vector.bn_aggr(mv, st)
                # (mean, E[x^2]) per channel
                st2 = sm.tile([P, 2], FP32, tag=f"st2{r}")
                nc.vector.tensor_copy(st2[:, 0:1], mv[:, 0:1])
                nc.vector.tensor_tensor(st2[:, 1:2], mv[:, 0:1], mv[:, 0:1],
                                        OP.mult)
                nc.vector.tensor_tensor(st2[:, 1:2], st2[:, 1:2], mv[:, 1:2],
                                        OP.add)
                # per-group (mean, E[x^2]) via PE combine
                G_ps = gnps.tile([NH, 2], FP32, tag="gps", name=f"gps{r}")
                nc.tensor.matmul(G_ps, amat_sb, st2, start=True, stop=True)
                stg = sm.tile([NH, 2], FP32, tag=f"stg{r}")
                nc.vector.tensor_copy(stg, G_ps)
                var = sm.tile([NH, 1], FP32, tag=f"var{r}")
                nc.vector.tensor_tensor(var, stg[:, 0:1], stg[:, 0:1], OP.mult)
                nc.vector.tensor_tensor(var, stg[:, 1:2], var, OP.subtract)
                nc.vector.tensor_scalar(var, var, 1e-5, None, OP.add)
                # rstd = rsqrt(var): 1/var seed + 2 Newton steps
                y = sm.tile([NH, 1], FP32, tag=f"rsy{r}")
                nc.vector.reciprocal_approx_fast(y, var)
                t_ = sm.tile([NH, 1], FP32, tag=f"rst{r}")
                for it in range(2):
                    nc.vector.tensor_tensor(t_, y, y, OP.mult)
                    nc.vector.tensor_tensor(t_, t_, var, OP.mult)
                    nc.vector.tensor_scalar(t_, t_, -0.5, 1.5, OP.mult, OP.add)
                    if it < 1:
                        nc.vector.tensor_tensor(y, y, t_, OP.mult)
                    else:
                        nc.vector.tensor_tensor(stg[:, 1:2], y, t_, OP.mult)
                # broadcast (mean, rstd) back to channels
                MR_ps = gnps.tile([P, 2], FP32, tag="mrps", name=f"mrps{r}")
                nc.tensor.matmul(MR_ps, imat_sb, stg, start=True, stop=True)
                mr = sm.tile([P, 2], FP32, tag=f"mr{r}")
                nc.vector.tensor_copy(mr, MR_ps)
                a_r = sm.tile([P, 1], FP32, tag=f"gn_a{r}")
                nc.vector.tensor_tensor(a_r, mr[:, 1:2], ggc_sb[:, r:r + 1],
                                        OP.mult)
                b_r = sm.tile([P, 1], FP32, tag=f"gn_b{r}")
                nc.vector.tensor_tensor(b_r, mr[:, 0:1], a_r, OP.mult)
                nc.vector.tensor_tensor(b_r, gbc_sb[:, r:r + 1], b_r,
                                        OP.subtract)
                nc.scalar.activation(h8_sb[:, r // 2, r % 2, :], x_sb[:, r, :],
                                     AF.Identity, bias=b_r, scale=a_r)

        # ------------- qkv + attention + proj (fp8 DoubleRow) -------------
        with nc.named_scope("attn"), \
             tc.tile_pool(name="bgps", bufs=2, space="PSUM") as bgps, \
             tc.tile_pool(name="spool", bufs=2, space="PSUM") as spool, \
             tc.tile_pool(name="opool", bufs=2, space="PSUM") as opool, \
             tc.tile_pool(name="epool", bufs=4) as epool, \
             tc.tile_pool(name="rpool", bufs=2) as rpool, \
             tc.tile_pool(name="outp", bufs=4) as outp:

            def qk_chain(dst8, w_sb, bcol, tr, sl, half):
                hs = 512 * half
                ps = bgps.tile([P, 512], FP32, tag="bg",
                               name=f"qk_{w_sb.name}_{tr}_{sl}_{half}")
                for kpr in range(2):
                    nc.tensor.matmul(ps, w_sb[:, kpr, :, tr, sl, :],
                                     h8_sb[:, kpr, :, hs:hs + 512],
                                     start=(kpr == 0), stop=(kpr == 1),
                                     perf_mode=DR)
                nc.vector.tensor_scalar(dst8[:, tr, sl, hs:hs + 512], ps,
                                        bcol[:, 2 * tr + sl:2 * tr + sl + 1],
                                        None, OP.add)

            def vt_chain(t):
                ps = bgps.tile([P, 512], FP32, tag="bg", name=f"vt{t}")
                for kpr in range(2):
                    nc.tensor.matmul(ps, h8_sb[:, kpr, :, P * t:P * t + P],
                                     wv8_sb[:, kpr, :, :],
                                     start=(kpr == 0), stop=(kpr == 1),
                                     perf_mode=DR)
                psv = ps.rearrange("p (h c) -> p h c", c=HD)
                if SCH_HEADS:
                    nc.vector.tensor_copy(vT16_sb[:, t, :, HD:P],
                                          psv[:, 0:len(SCH_HEADS), :])
                nc.vector.tensor_copy(vT8_sb[:, t, VT8_H0:NH, HD:P],
                                      psv[:, VT8_H0:NH, :])

            def xpb_task(r):
                nc.vector.tensor_scalar(xpb_sb[:, r, :], x_sb[:, r, :],
                                        pbc_sb[:, r:r + 1], None, OP.add)

            def proj_chain(r, half):
                hs = 512 * half
                ps = bgps.tile([P, 512], FP32, tag="bg", name=f"pj{r}_{half}")
                for opr in range(2):
                    nc.tensor.matmul(ps, pw8_sb[:, opr, :, r, :],
                                     O8_sb[:, opr, :, hs:hs + 512],
                                     start=(opr == 0), stop=(opr == 1),
                                     perf_mode=DR)
                o_sb = outp.tile([P, 512], FP32, tag="osb",
                                 name=f"osb{r}_{half}")
                nc.vector.tensor_tensor(o_sb, ps, xpb_sb[:, r, hs:hs + 512],
                                        OP.add)
                eng = nc.sync if (r + half) % 2 == 0 else nc.gpsimd
                eng.dma_start(io["out"][:, r, hs:hs + 512], o_sb)

            def proj_part(r, half):
                # opr=0 partial (heads 0-3) + xpb, staged to P1x
                hs = 512 * half
                ps = bgps.tile([P, 512], FP32, tag="bg", name=f"pp{r}_{half}")
                nc.tensor.matmul(ps, pw8_sb[:, 0, :, r, :],
                                 O8_sb[:, 0, :, hs:hs + 512],
                                 start=True, stop=True, perf_mode=DR)
                nc.vector.tensor_tensor(P1x_sb[:, r, hs:hs + 512], ps,
                                        xpb_sb[:, r, hs:hs + 512], OP.add)

            def proj_fin(r, half):
                hs = 512 * half
                ps = bgps.tile([P, 512], FP32, tag="bg", name=f"pf{r}_{half}")
                nc.tensor.matmul(ps, pw8_sb[:, 1, :, r, :],
                                 O8_sb[:, 1, :, hs:hs + 512],
                                 start=True, stop=True, perf_mode=DR)
                o_sb = outp.tile([P, 512], FP32, tag="osb",
                                 name=f"osb{r}_{half}")
                nc.vector.tensor_tensor(o_sb, ps, P1x_sb[:, r, hs:hs + 512],
                                        OP.add)
                eng = nc.sync if (r + half) % 2 == 0 else nc.gpsimd
                eng.dma_start(io["out"][:, r, hs:hs + 512], o_sb)

            # upfront: what head 0 (half 0) needs: q/k tiles tr=0
            # (heads 0-3) live in chains (0, sl, half)
            qk_chain(k8_sb, wk8_sb, bkc_sb, 0, 0, 0)
            qk_chain(k8_sb, wk8_sb, bkc_sb, 0, 1, 0)
            qk_chain(q8_sb, wq8_sb, bqc_sb, 0, 0, 0)
            qk_chain(q8_sb, wq8_sb, bqc_sb, 0, 1, 0)

            drip = {
                0: [(qk_chain, (k8_sb, wk8_sb, bkc_sb, 0, 0, 1)),
                    (vt_chain, (0,)), (vt_chain, (1,))],
                1: [(qk_chain, (k8_sb, wk8_sb, bkc_sb, 0, 1, 1)),
                    (vt_chain, (2,)), (vt_chain, (3,))],
                2: [(vt_chain, (4,)), (vt_chain, (5,))],
                3: [(vt_chain, (6,)), (vt_chain, (7,))],
                8: [(qk_chain, (k8_sb, wk8_sb, bkc_sb, 1, 0, 0)),
                    (qk_chain, (k8_sb, wk8_sb, bkc_sb, 1, 1, 0))],
                10: [(qk_chain, (k8_sb, wk8_sb, bkc_sb, 1, 0, 1)),
                     (qk_chain, (k8_sb, wk8_sb, bkc_sb, 1, 1, 1))],
                12: [(qk_chain, (q8_sb, wq8_sb, bqc_sb, 1, 0, 0)),
                     (qk_chain, (q8_sb, wq8_sb, bqc_sb, 1, 1, 0))],
                14: [(xpb_task, (0,))],
                16: [(xpb_task, (1,))],
                18: [(xpb_task, (2,))],
                20: [(xpb_task, (3,))],
                26: [(qk_chain, (q8_sb, wq8_sb, bqc_sb, 0, 0, 1)),
                     (qk_chain, (q8_sb, wq8_sb, bqc_sb, 0, 1, 1))],
                28: [(qk_chain, (q8_sb, wq8_sb, bqc_sb, 1, 0, 1)),
                     (qk_chain, (q8_sb, wq8_sb, bqc_sb, 1, 1, 1))],
                36: [(proj_chain, (0, 0))],
                38: [(proj_chain, (1, 0))],
                40: [(proj_chain, (2, 0))],
                42: [(proj_chain, (3, 0))],
                52: [(proj_part, (0, 1))],
                54: [(proj_part, (1, 1))],
                56: [(proj_part, (2, 1))],
                58: [(proj_part, (3, 1))],
            }

            O_ps_map = {}

            # a "group" is one key-tile PAIR of one (head, query-half):
            # 2 S matmuls -> 1 exp instr -> 1 DoubleRow AV matmul.
            # g = half*32 + h*4 + tq, tq in 0..3.
            def s_group(g):
                half, h, tq = g // 32, (g % 32) // 4, g % 4
                qr, hi = h // 4, h % 4
                S2 = spool.tile([P, 2, 512], FP32, tag="s2", name=f"s2_{g}")
                for j in range(2):
                    t = 2 * tq + j
                    nc.tensor.matmul(
                        S2[:, j, :],
                        k8_sb[32 * hi:32 * hi + 32, qr, :, P * t:P * t + P],
                        q8_sb[32 * hi:32 * hi + 32, qr, :,
                              512 * half:512 * half + 512],
                        start=True, stop=True, perf_mode=DR,
                        tile_position=(32 * hi, 0))
                if (h, tq) in SCH_SET:
                    E = epool.tile([P, 2, 512], I16, tag="e16", name=f"e_{g}")
                    nc.vector.tensor_scalar(E, S2, A_SCH, B_SCH,
                                            OP.mult, OP.add)
                else:
                    E = epool.tile([P, 2, 512], FP8, tag="e8", name=f"e_{g}")
                    nc.scalar.activation(E, S2, AF.Exp, bias=nbias)
                return E

            def av_group(g, E):
                half, h, tq = g // 32, (g % 32) // 4, g % 4
                if tq == 0:
                    O_ps_map[(h, half)] = opool.tile(
                        [P, 512], FP32, tag="o", name=f"o_{h}_{half}")
                O_ps = O_ps_map[(h, half)]
                if (h, tq) in SCH_SET:
                    Ebf = E.bitcast(BF16)
                    si = SCH_HEADS.index(h)
                    for j in range(2):
                        t = 2 * tq + j
                        nc.tensor.matmul(O_ps, vT16_sb[:, t, si, :],
                                         Ebf[:, j, :],
                                         start=(t == 0), stop=(t == 7),
                                         skip_group_check=True)
                else:
                    nc.tensor.matmul(
                        O_ps, vT8_sb[:, 2 * tq:2 * tq + 2, h, :], E,
                        start=(tq == 0), stop=(tq == 3), perf_mode=DR,
                        skip_group_check=True)
                if tq == 3:
                    epilogue(h, half)

            def epilogue(h, half):
                O_ps = O_ps_map.pop((h, half))
                Rh = rpool.tile([HD, 512], FP32, tag="rh",
                                name=f"rh{h}_{half}")
                nc.vector.reciprocal_approx_fast(Rh, O_ps[0:HD, :])
                p0 = HD * (h % 2)
                nc.vector.tensor_tensor(
                    O8_sb[p0:p0 + HD, h // 4, (h % 4) // 2,
                          512 * half:512 * half + 512],
                    O_ps[HD:P, :], Rh, OP.mult)

            pend = deque()
            for g in range(64):
                E = s_group(g)
                pend.append((g, E))
                while len(pend) > 1:
                    av_group(*pend.popleft())
                for fn, args in drip.pop(g, ()):
                    fn(*args)
            while pend:
                av_group(*pend.popleft())
            assert not drip

            with nc.named_scope("proj_tail"):
                for r in range(4):
                    proj_fin(r, 1)


_CACHE: dict = {}


def _build():
    if "nc" in _CACHE:
        return _CACHE["nc"]
    nc = bacc.Bacc("TRN2", target_bir_lowering=False, debug=False,
                   num_devices=NCORES)
    io = {
        "x": nc.dram_tensor("x", [P, 4, NT], FP32, kind="ExternalInput").ap(),
        "wq8": nc.dram_tensor("wq8", [P, 2, 2, 2, 2, P], FP8,
                              kind="ExternalInput").ap(),
        "wk8": nc.dram_tensor("wk8", [P, 2, 2, 2, 2, P], FP8,
                              kind="ExternalInput").ap(),
        "wv8": nc.dram_tensor("wv8", [P, 2, 2, C], FP8,
                              kind="ExternalInput").ap(),
        "pw8": nc.dram_tensor("pw8", [P, 2, 2, 4, P], FP8,
                              kind="ExternalInput").ap(),
        "bqc": nc.dram_tensor("bqc", [P, 4], FP32, kind="ExternalInput").ap(),
        "bkc": nc.dram_tensor("bkc", [P, 4], FP32, kind="ExternalInput").ap(),
        "pbc": nc.dram_tensor("pbc", [P, 4], FP32, kind="ExternalInput").ap(),
        "ggc": nc.dram_tensor("ggc", [P, 4], FP32, kind="ExternalInput").ap(),
        "gbc": nc.dram_tensor("gbc", [P, 4], FP32, kind="ExternalInput").ap(),
        "amat": nc.dram_tensor("amat", [P, NH], FP32,
                               kind="ExternalInput").ap(),
        "imat": nc.dram_tensor("imat", [NH, P], FP32,
                               kind="ExternalInput").ap(),
        "out": nc.dram_tensor("out", [P, 4, NT], FP32,
                              kind="ExternalOutput").ap(),
    }
    with tile.TileContext(nc) as tc:
        _emit(tc, io)
    nc.compile()
    _CACHE["nc"] = nc
    return nc


def _host_prep(inputs):
    x = np.ascontiguousarray(np.asarray(inputs["x"], dtype=np.float32))
    qkv_w = np.asarray(inputs["qkv_w"], dtype=np.float32)
    qkv_b = np.asarray(inputs["qkv_b"], dtype=np.float32)
    proj_w = np.asarray(inputs["proj_w"], dtype=np.float32)
    proj_b = np.asarray(inputs["proj_b"], dtype=np.float32)
    gn_scale = np.asarray(inputs["gn_scale"], dtype=np.float32)
    gn_bias = np.asarray(inputs["gn_bias"], dtype=np.float32)

    s = np.float32(1.0 / np.sqrt(HD))
    f8 = ml_dtypes.float8_e4m3

    # q/k output-channel reorder: oc(tr, sl, m) = (4*tr + m//32)*64 + sl*32
    # + m%32 -- head h on partitions 32*(h%4)..+32 with head-dim in 2 slots
    # so S matmuls can run DoubleRow with K=32 x 2 slots.
    tr_i = np.arange(2)[:, None, None]
    sl_i = np.arange(2)[None, :, None]
    m_i = np.arange(P)[None, None, :]
    oc_map = (4 * tr_i + m_i // 32) * 64 + sl_i * 32 + m_i % 32  # [2,2,128]

    def qk_weight(W):
        # -> [kpart(128), kpr(2), ksl(2), tr(2), sl(2), m(128)]
        Wr = W[oc_map.reshape(-1), :]               # rows reordered
        Wt = np.ascontiguousarray(Wr.T)             # [kc, oc']
        return Wt.reshape(2, 2, P, 2, 2, P).transpose(2, 0, 1, 3, 4, 5)

    def qk_bias(b):
        # -> [p(128), (tr*2+sl)(4)]
        bc = b[oc_map]                              # [2,2,128]
        return np.ascontiguousarray(bc.transpose(2, 0, 1).reshape(P, 4))

    Wq = qkv_w[0:C] * s
    Wk = qkv_w[C:2 * C]
    Wv = qkv_w[2 * C:3 * C]

    # wv: [kpart, kpr, ksl, oc(512)]
    wv8 = np.ascontiguousarray(Wv.T).reshape(2, 2, P, C).transpose(2, 0, 1, 3)

    # pw: O channel oc -> (opart, opr, osl): h = 4*opr + 2*osl + opart//64,
    # c = opart%64; lhsT[k=oc, m=o]: pw8[opart, opr, osl, r, m]
    PwT = np.ascontiguousarray(proj_w.T)            # [oc, o]
    pw8 = PwT.reshape(2, 2, 2, HD, 4, P).transpose(2, 3, 0, 1, 4, 5) \
        .reshape(P, 2, 2, 4, P)

    pb = (proj_b + proj_w @ qkv_b[2 * C:3 * C]).astype(np.float32)

    shared = {
        "wq8": np.ascontiguousarray(qk_weight(Wq)).astype(f8),
        "wk8": np.ascontiguousarray(qk_weight(Wk)).astype(f8),
        "wv8": np.ascontiguousarray(wv8).astype(f8),
        "pw8": np.ascontiguousarray(pw8).astype(f8),
        "bqc": qk_bias((qkv_b[0:C] * s).astype(np.float32)),
        "bkc": qk_bias(qkv_b[C:2 * C].astype(np.float32)),
        "pbc": np.ascontiguousarray(pb.reshape(4, P).T),
        "ggc": np.ascontiguousarray(gn_scale.reshape(4, P).T),
        "gbc": np.ascontiguousarray(gn_bias.reshape(4, P).T),
        # amat: [128, 8], 1/16 where channel p belongs to group j of its tile
        "amat": (np.kron(np.eye(NH, dtype=np.float32),
                         np.ones((GSZ, 1), np.float32)) / GSZ),
        # imat: [8, 128], 1.0 where channel p belongs to group j of its tile
        "imat": np.ascontiguousarray(np.kron(np.eye(NH, dtype=np.float32),
                                             np.ones((1, GSZ), np.float32))),
    }
    B = x.shape[0]
    in_maps = []
    for b in range(B):
        m = dict(shared)
        m["x"] = np.ascontiguousarray(
            x[b].reshape(4, P, NT).transpose(1, 0, 2))
        in_maps.append(m)
    return in_maps


def run(inputs, trace=False):
    nc = _build()
    in_maps = _host_prep(inputs)
    res = run_bass_kernel_spmd(nc, in_maps, list(range(NCORES)), trace=trace)
    out = np.stack([res.results[i]["out"] for i in range(NCORES)], axis=0)
    # [B, 128, 4, 1024] -> [B, 512, 32, 32]
    out = out.transpose(0, 2, 1, 3).reshape(len(in_maps), C, 32, 32)
    return out, res


def kernel(**inputs) -> np.ndarray:
    out, _ = run(inputs, trace=False)
    return out.astype(np.float32)
